# revision 8
# baseline (speedup 1.0000x reference)
"""Trainium2 Bass kernel for nn_CustomizedLinear (masked pathway linear).

out[b, p*768+e] = sum_d x[b,d] * (weight*mask.T)[p,d] * G[d,e] + bias[p]
with B=64, P=256, D=2000, E=768.

Sharding: tensor-parallel over the pathway dim P — 32 pathways per core on
8 cores; x and gene_embedding replicated.

Per-core compute: fp8e4 matmuls in DoubleRow perf mode (0.5 PE cycles per
output column and 250 contraction rows per instruction = 4x fp32r MACs).
Accuracy is recovered with a 3-term split: G is host-split into fp8 hi+lo
(G ~ G_hi + G_lo), and the strip s = x*wm (scaled x256, folded into x on
host, exact) is device-split into fp8 s_hi + s_lo. psum accumulates
s_hi@G_hi + s_hi@G_lo + s_lo@G_hi; the dropped s_lo@G_lo term and fp8
residuals land at ~3e-3 fro overall.

Per DoubleRow k-tile (250 genes = 125 partitions x 2 slots, slot
innermost so every elementwise operand is 2-byte with stride-1 inner dim,
which enables the DVE 2x_1p mode for the bf16 strip multiply):
DVE computes t = x*wm (bf16), ACT casts s_hi = fp8(t), DVE/Pool compute
s_lo = fp8(t - s_hi). Pathways are processed in groups; each pair of
pathways packs M=128 stationary columns, G streams in two N=384 chunks.
Input/output DMAs are split across both HWDGE rings.
"""
import sys

sys.path.insert(0, "/opt/trn_rl_repo")

import numpy as np
import ml_dtypes
from contextlib import ExitStack

import concourse.bacc as bacc
import concourse.tile as tile
import concourse.mybir as mybir
from concourse.bass_utils import run_bass_kernel_spmd

F32 = mybir.dt.float32
BF16 = mybir.dt.bfloat16
F8 = mybir.dt.float8e4
E4M3 = ml_dtypes.float8_e4m3
BF = ml_dtypes.bfloat16

N_CORES = 8
B = 64          # batch
D = 2000        # genes (contraction)
E = 768         # embedding
P_TOT = 256     # pathways
P_CORE = P_TOT // N_CORES        # 32 pathways per core
KT = 8                           # DoubleRow k-tiles
S = 2                            # DoubleRow slots per partition
KP = D // (KT * S)               # 125 partitions per k-tile
NCH = 2                          # N chunks per pair
NC_N = E // NCH                  # 384
SCALE = 256.0                    # strip prescale (folded into x on host)
BLK = (B + 2 * P_CORE) * S       # 256 bf16 cols per k-tile [x|w|m]
GW = 2 * S * E                   # fp8 cols per k-tile in g [hi/lo, slot, e]


def _build_program(group_sizes=(4, 4, 4, 4, 4, 4, 4, 2, 2),
                   pool_subs=(0, 0, 1, 1, 0, 1, 1, 1), head_kt=2,
                   n_warm=70):
    assert sum(group_sizes) == P_CORE
    nc = bacc.Bacc()
    xwm_d = nc.declare_dram_parameter("xwm", [KP, KT * BLK], BF16,
                                      isOutput=False)
    g_d = nc.declare_dram_parameter("g", [KP, KT * GW], F8, isOutput=False)
    bias_d = nc.declare_dram_parameter("bias", [2 * B, P_CORE // 2], F32,
                                       isOutput=False)
    out_d = nc.declare_dram_parameter("out", [B, P_CORE * E], F32,
                                      isOutput=True)

    def ring(i):
        return nc.sync if i % 2 == 0 else nc.scalar

    with tile.TileContext(nc) as tc, ExitStack() as ctx:
        const = ctx.enter_context(tc.tile_pool(name="const", bufs=1))
        strips = ctx.enter_context(tc.tile_pool(name="strips", bufs=4))
        outs = ctx.enter_context(tc.tile_pool(name="outs", bufs=4))
        psum = ctx.enter_context(tc.tile_pool(name="psum", bufs=8,
                                              space="PSUM"))

        # warm-up matmuls: keep the PE p-state ramping while input DMAs and
        # the first strip chain fill the pipeline
        if n_warm:
            warm = const.tile([KP, 768], F8)
            nc.vector.memset(warm[:], 0)
            wl = warm[:, :256].rearrange("d (s m) -> d s m", s=S)
            wr = warm[:, 512:768].rearrange("d (s n) -> d s n", s=S)
            wps = psum.tile([2 * B, NC_N], F32, tag="ps", name="warm")
            for _ in range(n_warm):
                nc.tensor.matmul(wps[:, :128], wl, wr, start=True, stop=True,
                                 perf_mode=mybir.MatmulPerfMode.DoubleRow)

        # x/w/m head k-tiles first in the sync queue (critical path)
        xwm_h = const.tile([KP, head_kt * BLK], BF16)
        nc.sync.dma_start(out=xwm_h[:], in_=xwm_d[:, :head_kt * BLK])
        xwm_t = const.tile([KP, (KT - head_kt) * BLK], BF16)
        bias_t = const.tile([2 * B, P_CORE // 2], F32)

        def xv(k):  # x view for k-tile: [KP, B, S]
            tl, kk = (xwm_h, k) if k < head_kt else (xwm_t, k - head_kt)
            return tl[:, kk * BLK:kk * BLK + B * S].rearrange(
                "d (b s) -> d b s", s=S)

        def wmv(tl, kk, n_kt):  # w/m views inside a block range
            w4 = tl[:].rearrange("d (k c) -> d k c", k=n_kt)[
                :, :, B * S:(B + P_CORE) * S].rearrange(
                "d k (p s) -> d k p s", s=S)
            m4 = tl[:].rearrange("d (k c) -> d k c", k=n_kt)[
                :, :, (B + P_CORE) * S:].rearrange(
                "d k (p s) -> d k p s", s=S)
            return w4, m4

        # masked weights, bf16: wm[d, k, p, s]
        wm_t = const.tile([KP, KT * P_CORE * S], BF16)
        wm4 = wm_t[:].rearrange("d (k p s) -> d k p s", k=KT, s=S)
        w4h, m4h = wmv(xwm_h, 0, head_kt)
        nc.vector.tensor_mul(wm4[:, :head_kt], w4h, m4h)

        # G stream: fp8 hi/lo interleaved per k-tile.
        # Queue plan (DMA engines serialize globally, so order by need time):
        #   sync:   xwm_h, g_k0, xwm_t, g_k2-3
        #   scalar: g_k1, g_k4-5, g_k6-7, bias
        g_sb = const.tile([KP, KT * GW], F8)
        g5 = g_sb[:].rearrange("d (k h s e) -> d k h s e", k=KT, h=2, s=S)

        def g_dma(eng, ka, kb):
            eng.dma_start(out=g_sb[:, ka * GW:kb * GW],
                          in_=g_d[:, ka * GW:kb * GW])

        g_dma(nc.sync, 0, 1)
        g_dma(nc.scalar, 1, 2)
        nc.sync.dma_start(out=xwm_t[:], in_=xwm_d[:, head_kt * BLK:])
        w4t, m4t = wmv(xwm_t, 0, KT - head_kt)
        nc.vector.tensor_mul(wm4[:, head_kt:], w4t, m4t)
        g_dma(nc.scalar, 4, 6)
        g_dma(nc.sync, 2, 4)
        g_dma(nc.scalar, 6, 8)
        nc.scalar.dma_start(out=bias_t[:], in_=bias_d[:])

        out_p = out_d[:].rearrange("b (p e) -> p b e", p=P_CORE)

        odma = 0
        p_start = 0
        n_groups = len(group_sizes)
        for gi, gp in enumerate(group_sizes):
            npair = gp // 2
            ps = [psum.tile([2 * B, NC_N], F32, tag="ps",
                            name=f"ps{gi}_{i}") for i in range(npair * NCH)]
            for k in range(KT):
                fe = gp * B * S
                t = strips.tile([KP, fe], BF16, tag=f"t{gp}",
                                name=f"t{gi}_{k}")
                t4 = t[:].rearrange("d (p b s) -> d p b s", p=gp, s=S)
                x_bc = xv(k).unsqueeze(1).broadcast_to([KP, gp, B, S])
                w_bc = (wm4[:, k, p_start:p_start + gp]
                        .unsqueeze(2).broadcast_to([KP, gp, B, S]))
                nc.vector.tensor_mul(t4, x_bc, w_bc)
                # s_hi/s_lo written slot-major (the dual-fp8 ldweights ISA
                # requires contiguous stationary columns); t is read through
                # a strided view at no engine cost
                t4r = t[:].rearrange("d (p b s) -> d s p b", p=gp, s=S)
                s_hi = strips.tile([KP, fe], F8, tag=f"sh{gp}",
                                   name=f"sh{gi}_{k}")
                sh4 = s_hi[:].rearrange("d (s p b) -> d s p b", s=S, p=gp)
                nc.scalar.activation(sh4, t4r,
                                     mybir.ActivationFunctionType.Identity)
                s_lo = strips.tile([KP, fe], F8, tag=f"sl{gp}",
                                   name=f"sl{gi}_{k}")
                sl4 = s_lo[:].rearrange("d (s p b) -> d s p b", s=S, p=gp)
                sub_eng = nc.gpsimd if pool_subs[k] else nc.vector
                sub_eng.tensor_sub(sl4, t4r, sh4)

                hi3 = s_hi[:].rearrange("d (s m) -> d s m", s=S)
                lo3 = s_lo[:].rearrange("d (s m) -> d s m", s=S)
                for pr in range(npair):
                    lhs_hi = hi3[:, :, 128 * pr:128 * (pr + 1)]
                    lhs_lo = lo3[:, :, 128 * pr:128 * (pr + 1)]
                    for term, lhs, h in ((0, lhs_hi, 0), (1, lhs_hi, 1),
                                         (2, lhs_lo, 0)):
                        for n in range(NCH):
                            nc.tensor.matmul(
                                ps[NCH * pr + n][:],
                                lhs,
                                g5[:, k, h, :, NC_N * n:NC_N * (n + 1)],
                                start=(k == 0 and term == 0),
                                stop=(k == KT - 1 and term == 2),
                                perf_mode=mybir.MatmulPerfMode.DoubleRow,
                            )
            for pr in range(npair):
                pg = p_start // 2 + pr
                last = gi >= n_groups - 2
                o = outs.tile([2 * B, E], F32, tag="o", name=f"o{gi}_{pr}")
                p0 = 2 * pg
                for n in range(NCH):
                    osl = o[:, NC_N * n:NC_N * (n + 1)]
                    if n == 0:
                        nc.scalar.activation(
                            osl, ps[NCH * pr + n][:],
                            mybir.ActivationFunctionType.Identity,
                            bias=bias_t[:, pg:pg + 1], scale=1.0 / SCALE,
                        )
                    else:
                        # DVE drain: out = psum * (1/SCALE) + bias
                        nc.vector.scalar_tensor_tensor(
                            osl, ps[NCH * pr + n][:], 1.0 / SCALE,
                            bias_t[:, pg:pg + 1].broadcast_to([2 * B, NC_N]),
                            op0=mybir.AluOpType.mult,
                            op1=mybir.AluOpType.add,
                        )
                    if last:
                        dst = out_p[p0:p0 + 2, :, NC_N * n:NC_N * (n + 1)]
                        ring(odma).dma_start(out=dst, in_=osl)
                        odma += 1
                if not last:
                    ring(odma).dma_start(out=out_p[p0:p0 + 2, :, :], in_=o[:])
                    odma += 1
            p_start += gp

    nc.finalize()
    return nc


_NC_CACHE = None


def _get_program():
    global _NC_CACHE
    if _NC_CACHE is None:
        _NC_CACHE = _build_program()
    return _NC_CACHE


def _kpack(a):
    """[D, X] -> [KP, KT, S, X]: row d of k-tile k, slot s = gene k*250+s*125+d."""
    x = a.shape[1]
    return a.reshape(KT, S, KP, x).transpose(2, 0, 1, 3)


def _make_in_maps(x, weight, bias, mask, gene_embedding):
    # x scaled by 256 (exact in bf16), transposed to [D, B]
    xs = _kpack((x * SCALE).T.astype(BF))          # [KP, KT, S, B]
    xs = xs.transpose(0, 1, 3, 2)                  # [KP, KT, B, S]
    g32 = gene_embedding.astype(np.float32)
    g_hi = g32.astype(E4M3)
    g_lo = (g32 - g_hi.astype(np.float32)).astype(E4M3)
    gh = _kpack(g_hi)                              # [KP, KT, S, E]
    gl = _kpack(g_lo)
    g_pack = np.ascontiguousarray(
        np.stack([gh, gl], axis=2)                 # [KP, KT, 2, S, E]
    ).reshape(KP, KT * GW)

    in_maps = []
    for c in range(N_CORES):
        sl = slice(P_CORE * c, P_CORE * (c + 1))
        wp = _kpack(weight[sl].T.astype(BF)).transpose(0, 1, 3, 2)  # [KP,KT,P,S]
        mp = _kpack(mask[:, sl].astype(BF)).transpose(0, 1, 3, 2)
        xwm = np.ascontiguousarray(np.concatenate(
            [xs.reshape(KP, KT, B * S),
             wp.reshape(KP, KT, P_CORE * S),
             mp.reshape(KP, KT, P_CORE * S)], axis=2)).reshape(KP, KT * BLK)
        b_c = bias[sl]
        bias_sb = np.ascontiguousarray(
            np.repeat(b_c.reshape(P_CORE // 2, 2), B, axis=1).T.astype(
                np.float32))
        in_maps.append({"xwm": xwm, "g": g_pack, "bias": bias_sb})
    return in_maps


def kernel(x, weight, bias, mask, gene_embedding, _want_results=False, **_):
    x = np.ascontiguousarray(x, dtype=np.float32)
    weight = np.ascontiguousarray(weight, dtype=np.float32)
    bias = np.ascontiguousarray(bias, dtype=np.float32)
    mask = np.ascontiguousarray(mask, dtype=np.float32)
    g = np.ascontiguousarray(gene_embedding, dtype=np.float32)

    in_maps = _make_in_maps(x, weight, bias, mask, g)
    nc = _get_program()
    res = run_bass_kernel_spmd(nc, in_maps, list(range(N_CORES)))
    out = np.concatenate([r["out"] for r in res.results], axis=1)
    if _want_results:
        return out, res
    return out


# revision 14
# speedup vs baseline: 1.2159x; 1.2159x over previous
"""Trainium2 Bass kernel for nn_CustomizedLinear (masked pathway linear).

out[b, p*768+e] = sum_d x[b,d] * (weight*mask.T)[p,d] * G[d,e] + bias[p]
with B=64, P=256, D=2000, E=768.

Sharding: tensor-parallel over the pathway dim P — 32 pathways per core on
8 cores; x and gene_embedding replicated.

Per-core compute: fp8e4 matmuls in DoubleRow perf mode (0.5 PE cycles per
output column and 250 contraction rows per instruction = 4x fp32r MACs).
Accuracy is recovered with a 3-term split: G is host-split into fp8 hi+lo
(G ~ G_hi + G_lo), and the strip s = x*wm (scaled x256, folded into x on
host, exact) is device-split into fp8 s_hi + s_lo. psum accumulates
s_hi@G_hi + s_hi@G_lo + s_lo@G_hi; the dropped s_lo@G_lo term and fp8
residuals land at ~3e-3 fro overall.

Per DoubleRow k-tile (250 genes = 125 partitions x 2 slots, slot
innermost so every elementwise operand is 2-byte with stride-1 inner dim,
which enables the DVE 2x_1p mode for the bf16 strip multiply):
DVE computes t = x*wm (bf16), ACT casts s_hi = fp8(t), DVE/Pool compute
s_lo = fp8(t - s_hi). Pathways are processed in groups; each pair of
pathways packs M=128 stationary columns, G streams in two N=384 chunks.
Input/output DMAs are split across both HWDGE rings.
"""
import sys

sys.path.insert(0, "/opt/trn_rl_repo")

import numpy as np
import ml_dtypes
from contextlib import ExitStack

import concourse.bacc as bacc
import concourse.tile as tile
import concourse.mybir as mybir
from concourse.bass_utils import run_bass_kernel_spmd

F32 = mybir.dt.float32
BF16 = mybir.dt.bfloat16
F8 = mybir.dt.float8e4
E4M3 = ml_dtypes.float8_e4m3
BF = ml_dtypes.bfloat16

N_CORES = 8
B = 64          # batch
D = 2000        # genes (contraction)
E = 768         # embedding
P_TOT = 256     # pathways
P_CORE = P_TOT // N_CORES        # 32 pathways per core
KT = 8                           # DoubleRow k-tiles
S = 2                            # DoubleRow slots per partition
KP = D // (KT * S)               # 125 partitions per k-tile
NCH = 2                          # N chunks per pair
NC_N = E // NCH                  # 384
SCALE = 256.0                    # strip prescale (folded into x on host)
BLK = (B + P_CORE) * S           # 192 bf16 cols per k-tile [x|wm]
GW = 2 * S * E                   # fp8 cols per k-tile in g (hi+lo)
GWH = S * E                      # fp8 cols per k-tile per hi/lo block
GLAG = 2                         # default hi@G_lo deferral (k-tiles)


def _build_program(group_sizes=(4,) * 8,
                   sub_engs=("v", "v", "p", "v", "p", "p", "v", "p"),
                   cast_engs=("a", "a", "v", "a", "v", "a", "v", "a"),
                   head_kt=2, n_warm=130, tail_dve_drain=True, lag=2,
                   outs_bufs=4, warm_own=True, strip_bufs=8, glag=1,
                   defer_lo=1,
                   dma_plan=(("b", "x", 0, 2), ("a", "gh", 0, 2),
                             ("b", "x", 2, 8), ("a", "gh", 2, 4),
                             ("b", "gl", 0, 2), ("a", "gh", 4, 6),
                             ("b", "gl", 2, 4), ("a", "gh", 6, 8),
                             ("b", "gl", 4, 6), ("a", "gl", 6, 8),
                             ("b", "bias", 0, 0))):
    assert sum(group_sizes) == P_CORE
    nc = bacc.Bacc()
    xwm_d = nc.declare_dram_parameter("xwm", [KP, KT * BLK], BF16,
                                      isOutput=False)
    g_d = nc.declare_dram_parameter("g", [KP, KT * GW], F8, isOutput=False)
    bias_d = nc.declare_dram_parameter("bias", [2 * B, P_CORE // 2], F32,
                                       isOutput=False)
    out_d = nc.declare_dram_parameter("out", [B, P_CORE * E], F32,
                                      isOutput=True)

    def ring(i):
        return nc.sync if i % 2 == 0 else nc.scalar

    with tile.TileContext(nc) as tc, ExitStack() as ctx:
        const = ctx.enter_context(tc.tile_pool(name="const", bufs=1))
        strips = ctx.enter_context(tc.tile_pool(name="strips", bufs=strip_bufs))
        outs = ctx.enter_context(tc.tile_pool(name="outs", bufs=outs_bufs))
        psum = ctx.enter_context(tc.tile_pool(name="psum", bufs=8,
                                              space="PSUM"))


        # Input DMA plan: list of (ring, kind, lo, hi) executed in order;
        # ring "b"=sync(SP), "a"=scalar(ACT). DMA engines serialize globally
        # and each ring is FIFO, so the plan encodes need-time priority.
        xwm_h = const.tile([KP, head_kt * BLK], BF16)
        xwm_t = const.tile([KP, (KT - head_kt) * BLK], BF16)
        bias_t = const.tile([2 * B, P_CORE // 2], F32)
        g_sb = const.tile([KP, KT * GW], F8)
        g5 = g_sb[:].rearrange("d (h k s e) -> d h k s e", h=2, k=KT, s=S)

        def xv(k):  # x view for k-tile: [KP, B, S]
            tl, kk = (xwm_h, k) if k < head_kt else (xwm_t, k - head_kt)
            return tl[:, kk * BLK:kk * BLK + B * S].rearrange(
                "d (b s) -> d b s", s=S)

        def wmv(k):  # wm view for k-tile: [KP, P_CORE, S]
            tl, kk = (xwm_h, k) if k < head_kt else (xwm_t, k - head_kt)
            return tl[:, kk * BLK + B * S:(kk + 1) * BLK].rearrange(
                "d (p s) -> d p s", s=S)

        for ring_id, kind, lo, hi in dma_plan:
            eng = nc.sync if ring_id == "b" else nc.scalar
            if kind == "x":
                if lo == 0:
                    eng.dma_start(out=xwm_h[:],
                                  in_=xwm_d[:, :head_kt * BLK])
                else:
                    a = lo - head_kt
                    b_ = hi - head_kt
                    eng.dma_start(out=xwm_t[:, a * BLK:b_ * BLK],
                                  in_=xwm_d[:, lo * BLK:hi * BLK])
            elif kind in ("gh", "gl"):
                off = 0 if kind == "gh" else KT * GWH
                eng.dma_start(out=g_sb[:, off + lo * GWH:off + hi * GWH],
                              in_=g_d[:, off + lo * GWH:off + hi * GWH])
            else:
                eng.dma_start(out=bias_t[:], in_=bias_d[:])

        out_p = out_d[:].rearrange("b (p e) -> p b e", p=P_CORE)

        # software-pipelined emission: strip chains run LAG (group,kt) jobs
        # ahead of their matmuls so group boundaries don't stall the PE
        p_starts = []
        acc = 0
        for gp in group_sizes:
            p_starts.append(acc)
            acc += gp
        jobs = [(gi, k) for gi in range(len(group_sizes)) for k in range(KT)]
        strip_tiles = {}
        ps_tiles = {}
        odma = [0]

        def emit_strip(gi, k):
            gp = group_sizes[gi]
            p0 = p_starts[gi]
            fe = gp * B * S
            t = strips.tile([KP, fe], BF16, tag=f"t{gp}", name=f"t{gi}_{k}")
            t4 = t[:].rearrange("d (p b s) -> d p b s", p=gp, s=S)
            x_bc = xv(k).unsqueeze(1).broadcast_to([KP, gp, B, S])
            w_bc = (wmv(k)[:, p0:p0 + gp]
                    .unsqueeze(2).broadcast_to([KP, gp, B, S]))
            nc.vector.tensor_mul(t4, x_bc, w_bc)
            t4r = t[:].rearrange("d (p b s) -> d s p b", p=gp, s=S)
            s_hi = strips.tile([KP, fe], F8, tag=f"sh{gp}", name=f"sh{gi}_{k}")
            sh4 = s_hi[:].rearrange("d (s p b) -> d s p b", s=S, p=gp)
            if cast_engs[k] == "a":
                nc.scalar.activation(sh4, t4r,
                                     mybir.ActivationFunctionType.Identity)
            else:
                nc.vector.tensor_copy(sh4, t4r)
            s_lo = strips.tile([KP, fe], F8, tag=f"sl{gp}", name=f"sl{gi}_{k}")
            sl4 = s_lo[:].rearrange("d (s p b) -> d s p b", s=S, p=gp)
            sub_eng = nc.gpsimd if sub_engs[k] == "p" else nc.vector
            sub_eng.tensor_sub(sl4, t4r, sh4)
            strip_tiles[gi, k] = (s_hi, s_lo)

        def mm_one(ps_t, lhs, k, h, n, start, stop):
            nc.tensor.matmul(
                ps_t[:], lhs, g5[:, h, k, :, NC_N * n:NC_N * (n + 1)],
                start=start, stop=stop,
                perf_mode=mybir.MatmulPerfMode.DoubleRow,
            )

        def emit_mms(gi, k):
            gp = group_sizes[gi]
            npair = gp // 2
            if k == 0 and gi not in ps_tiles:
                ps_tiles[gi] = [psum.tile([2 * B, NC_N], F32, tag="ps",
                                          name=f"ps{gi}_{i}")
                                for i in range(npair * NCH)]
            ps = ps_tiles[gi]

            def lhs_of(kk, which, pr):
                s_hi, s_lo = strip_tiles[gi, kk]
                t3 = (s_hi if which == 0 else s_lo)[:].rearrange(
                    "d (s m) -> d s m", s=S)
                return t3[:, :, 128 * pr:128 * (pr + 1)]

            # hi@G_hi and lo@G_hi for k-tile k now; hi@G_lo deferred GLAG
            # k-tiles so the G_lo DMA stream can trail the G_hi stream
            sched = [(0, k, 0)]
            if k >= defer_lo:
                sched.append((1, k - defer_lo, 0))
            if k >= glag:
                sched.append((0, k - glag, 1))
            if k == KT - 1:
                for kk in range(KT - defer_lo, KT):
                    sched.append((1, kk, 0))
                for kk in range(KT - glag, KT):
                    sched.append((0, kk, 1))
            for pr in range(npair):
                for idx, (which, kk, h) in enumerate(sched):
                    for n in range(NCH):
                        mm_one(ps[NCH * pr + n], lhs_of(kk, which, pr),
                               kk, h, n,
                               k == 0 and idx == 0,
                               k == KT - 1 and idx == len(sched) - 1)
            if k == KT - 1:
                for kk in range(KT):
                    del strip_tiles[gi, kk]

        def emit_tail_group(gi):
            # pair-major: pair0's psums finish a full kt-pass early so their
            # drain+DMA pipeline overlaps pair1's matmuls; pair1 output is
            # n-split so the last transfer is half-size
            gp = group_sizes[gi]
            npair = gp // 2
            ps_tiles[gi] = [psum.tile([2 * B, NC_N], F32, tag="ps",
                                      name=f"ps{gi}_{i}")
                            for i in range(npair * NCH)]
            ps = ps_tiles[gi]
            for pr in range(npair):
                pg = p_starts[gi] // 2 + pr
                p0 = 2 * pg
                o = outs.tile([2 * B, E], F32, tag="o", name=f"o{gi}_{pr}")
                def lhs_of(kk, which):
                    s_hi, s_lo = strip_tiles[gi, kk]
                    t3 = (s_hi if which == 0 else s_lo)[:].rearrange(
                        "d (s m) -> d s m", s=S)
                    return t3[:, :, 128 * pr:128 * (pr + 1)]

                for k in range(KT):
                    sched = [(0, k, 0)]
                    if k >= defer_lo:
                        sched.append((1, k - defer_lo, 0))
                    if k >= glag:
                        sched.append((0, k - glag, 1))
                    if k == KT - 1:
                        for kk in range(KT - defer_lo, KT):
                            sched.append((1, kk, 0))
                        for kk in range(KT - glag, KT):
                            sched.append((0, kk, 1))
                    for idx, (which, kk, h) in enumerate(sched):
                        for n in range(NCH):
                            mm_one(ps[NCH * pr + n], lhs_of(kk, which),
                                   kk, h, n,
                                   k == 0 and idx == 0,
                                   k == KT - 1 and idx == len(sched) - 1)
                for n in range(NCH):
                    osl = o[:, NC_N * n:NC_N * (n + 1)]
                    if n == 0 or not tail_dve_drain:
                        nc.scalar.activation(
                            osl, ps[NCH * pr + n][:],
                            mybir.ActivationFunctionType.Identity,
                            bias=bias_t[:, pg:pg + 1], scale=1.0 / SCALE,
                        )
                    else:
                        nc.vector.scalar_tensor_tensor(
                            osl, ps[NCH * pr + n][:], 1.0 / SCALE,
                            bias_t[:, pg:pg + 1].broadcast_to([2 * B, NC_N]),
                            op0=mybir.AluOpType.mult,
                            op1=mybir.AluOpType.add,
                        )
                    dst = out_p[p0:p0 + 2, :, NC_N * n:NC_N * (n + 1)]
                    ring(odma[0]).dma_start(out=dst, in_=osl)
                    odma[0] += 1
            for kk in range(KT):
                del strip_tiles[gi, kk]
            del ps_tiles[gi]

        def emit_drains(gi):
            gp = group_sizes[gi]
            npair = gp // 2
            ps = ps_tiles.pop(gi)
            for pr in range(npair):
                pg = p_starts[gi] // 2 + pr
                o = outs.tile([2 * B, E], F32, tag="o", name=f"o{gi}_{pr}")
                p0 = 2 * pg
                for n in range(NCH):
                    osl = o[:, NC_N * n:NC_N * (n + 1)]
                    nc.scalar.activation(
                        osl, ps[NCH * pr + n][:],
                        mybir.ActivationFunctionType.Identity,
                        bias=bias_t[:, pg:pg + 1], scale=1.0 / SCALE,
                    )
                ring(odma[0]).dma_start(out=out_p[p0:p0 + 2, :, :], in_=o[:])
                odma[0] += 1

        # group0 psum tiles double as the warm-up target: warm matmuls keep
        # the PE p-state ramping while input DMAs fill the pipeline, and the
        # first real matmul's start=True reset discards the warm garbage
        if n_warm:
            if not warm_own:
                ps_tiles[0] = [psum.tile([2 * B, NC_N], F32, tag="ps",
                                         name=f"ps0_{i}")
                               for i in range((group_sizes[0] // 2) * NCH)]
            warm = const.tile([KP, 256], F8)
            nc.gpsimd.memset(warm[:], 0)
            wl = warm[:].rearrange("d (s m) -> d s m", s=S)
            wr = warm[:].rearrange("d (s n) -> d s n", s=S)
            wps = (psum.tile([2 * B, NC_N], F32, tag="ps", name="warm")
                   if warm_own else ps_tiles[0][0])
            for _ in range(n_warm):
                nc.tensor.matmul(wps[:, :128], wl, wr, start=True, stop=True,
                                 perf_mode=mybir.MatmulPerfMode.DoubleRow)

        n_g = len(group_sizes)
        for i in range(len(jobs) + lag):
            if i < len(jobs):
                emit_strip(*jobs[i])
            j = i - lag
            if j >= 0:
                gi, k = jobs[j]
                if gi == n_g - 1:
                    if k == KT - 1:
                        emit_tail_group(gi)
                    continue
                emit_mms(gi, k)
                if k == KT - 1:
                    emit_drains(gi)

    nc.finalize()
    return nc


_NC_CACHE = None


def _get_program():
    global _NC_CACHE
    if _NC_CACHE is None:
        _NC_CACHE = _build_program()
    return _NC_CACHE


def _kpack(a):
    """[D, X] -> [KP, KT, S, X]: row d of k-tile k, slot s = gene k*250+s*125+d."""
    x = a.shape[1]
    return a.reshape(KT, S, KP, x).transpose(2, 0, 1, 3)


def _make_in_maps(x, weight, bias, mask, gene_embedding):
    # x scaled by 256 (exact in bf16), transposed to [D, B]
    xs = _kpack((x * SCALE).T.astype(BF))          # [KP, KT, S, B]
    xs = xs.transpose(0, 1, 3, 2)                  # [KP, KT, B, S]
    g32 = gene_embedding.astype(np.float32)
    g_hi = g32.astype(E4M3)
    g_lo = (g32 - g_hi.astype(np.float32)).astype(E4M3)
    gh = _kpack(g_hi)                              # [KP, KT, S, E]
    gl = _kpack(g_lo)
    g_pack = np.ascontiguousarray(
        np.stack([gh, gl], axis=1)                 # [KP, 2, KT, S, E]
    ).reshape(KP, KT * GW)

    wm_full = (weight * mask.T).astype(BF)      # mask folded (init-time in
    in_maps = []                                 # the original module)
    for c in range(N_CORES):
        sl = slice(P_CORE * c, P_CORE * (c + 1))
        wp = _kpack(wm_full[sl].T).transpose(0, 1, 3, 2)   # [KP,KT,P,S]
        xwm = np.ascontiguousarray(np.concatenate(
            [xs.reshape(KP, KT, B * S),
             wp.reshape(KP, KT, P_CORE * S)], axis=2)).reshape(KP, KT * BLK)
        b_c = bias[sl]
        bias_sb = np.ascontiguousarray(
            np.repeat(b_c.reshape(P_CORE // 2, 2), B, axis=1).T.astype(
                np.float32))
        in_maps.append({"xwm": xwm, "g": g_pack, "bias": bias_sb})
    return in_maps


def kernel(x, weight, bias, mask, gene_embedding, _want_results=False, **_):
    x = np.ascontiguousarray(x, dtype=np.float32)
    weight = np.ascontiguousarray(weight, dtype=np.float32)
    bias = np.ascontiguousarray(bias, dtype=np.float32)
    mask = np.ascontiguousarray(mask, dtype=np.float32)
    g = np.ascontiguousarray(gene_embedding, dtype=np.float32)

    in_maps = _make_in_maps(x, weight, bias, mask, g)
    nc = _get_program()
    res = run_bass_kernel_spmd(nc, in_maps, list(range(N_CORES)))
    out = np.concatenate([r["out"] for r in res.results], axis=1)
    if _want_results:
        return out, res
    return out

